# revision 1
# baseline (speedup 1.0000x reference)
"""Continuous-time RNN kernel for Trainium2 (8 NeuronCores, Bass/Tile).

Math (per reference):
    ih    = x @ W_ih.T + b_ih                     # time-invariant drive
    decay = exp(-dt / tau),  dt = 0.1
    10x:  h = decay * h + (1 - decay) * tanh(ih + h @ W_hh.T + b_hh)

Strategy (fp8 DoubleRow):
  - Data-parallel over batch: 4096 rows -> 8 cores x 512.
  - Reformulated recurrence in scaled-pre space (d scalar since tau==1):
        P_0     = (A@q(h0) + A@q(h0-q(h0)) + zR0 + C) * 1/(1-d)   [P = 2^s*pre]
        P_{r+1} = d*P_r + Cz + A@u_r
        u_r     = tanh(2^-s * P_r)    written directly as fp8 pairs by ACT
        acc_r   = d*acc_{r-1} + u_r   on gpsimd,  out = (1-d)*acc_9
    with A = 2^s*(1-d)*W_hh in fp8 e4m3 and a residual correction
    R = 2^g*(A - fp8(A)) in fp8 whose product R@moving is computed at
    rounds 0 and 1 only (u drifts ~8%/round so the cached correction
    stays accurate; round 0 matters most due to 1/(1-d) amplification)
    and folded into the additive constant C.
  - Matmuls use perf_mode=DoubleRow: contraction 256 per instruction
    (operands packed [128, 2, F] pair slabs), ~1.8x bf16 throughput.
  - Epilogue per (round, j): 2 DVE ops (scalar_tensor_tensor + add),
    1 ACT op (tanh, fp8 out), 1 gpsimd op (acc update) — under the
    PE's 8 matmuls (~1.73us).  Output staged in the dead P tiles.
  - Simulated end-to-end rel err ~5e-3 absmax (threshold 2e-2).
"""

import numpy as np
import ml_dtypes

H = 2048
I = 1024
B_TOTAL = 4096
N_CORES = 8
B = B_TOTAL // N_CORES  # 512 per-core batch shard
KJ = H // 128  # 16 output chunks of the hidden dim
K2 = KJ // 2  # 8 DoubleRow contraction pair-chunks
KI = I // 128  # 8 contraction chunks of the input dim
NUM_STEPS = 10
DT = 0.1
S_POW = 8  # weight scale 2^8
GAMMA = 5  # residual extra scale 2^5
REFRESH = (0,)  # rounds recomputing the weight-residual correction

_NC_CACHE = {}


def _build_nc(omdd_imm=None):
    import concourse.mybir as mybir
    import concourse.tile as tile
    from concourse import bacc

    f32 = mybir.dt.float32
    bf16 = mybir.dt.bfloat16
    f8 = mybir.dt.float8e4
    Tanh = mybir.ActivationFunctionType.Tanh
    Alu = mybir.AluOpType
    DR = mybir.MatmulPerfMode.DoubleRow

    nc = bacc.Bacc(None, target_bir_lowering=False, debug=False)

    x_t = nc.declare_dram_parameter("x_t", [I, B], bf16, isOutput=False)
    wih = nc.declare_dram_parameter("wih", [I, H], bf16, isOutput=False)
    a8 = nc.declare_dram_parameter("a8", [128, KJ * K2 * 2 * 128], f8, isOutput=False)
    r8 = nc.declare_dram_parameter("r8", [128, KJ * K2 * 2 * 128], f8, isOutput=False)
    h8p = nc.declare_dram_parameter("h8p", [128, K2 * 2 * B], f8, isOutput=False)
    h8r = nc.declare_dram_parameter("h8r", [128, K2 * 2 * B], f8, isOutput=False)
    # per-partition vectors: [dec|omd|i1d|bsm|dv|om2|omdd], each [128, KJ]
    vecs = nc.declare_dram_parameter("vecs", [128, 7 * KJ], f32, isOutput=False)
    hout = nc.declare_dram_parameter("hout", [H, B], f32, isOutput=True)

    a8_r = a8[:].rearrange("p (j k two q) -> j p k two q", j=KJ, k=K2, two=2)
    r8_r = r8[:].rearrange("p (j k two q) -> j p k two q", j=KJ, k=K2, two=2)
    h8p_r = h8p[:].rearrange("p (k two b) -> p k two b", k=K2, two=2)
    h8r_r = h8r[:].rearrange("p (k two b) -> p k two b", k=K2, two=2)
    wih_r = wih[:].rearrange("(k p) j -> k p j", p=128)
    xt_r = x_t[:].rearrange("(i p) b -> p i b", p=128)  # [128, KI, B]
    ho_r = hout[:].rearrange("(k p) b -> k p b", p=128)

    with tile.TileContext(nc) as tc:
        with (
            tc.tile_pool(name="vecp", bufs=1) as vecp,
            tc.tile_pool(name="a8p", bufs=1) as a8p,
            tc.tile_pool(name="r8p", bufs=1) as r8p,
            tc.tile_pool(name="h8pool", bufs=1) as h8pool,
            tc.tile_pool(name="cp", bufs=1) as cp,
            tc.tile_pool(name="accp", bufs=1) as accp,
            tc.tile_pool(name="scr", bufs=4) as scr,
            tc.tile_pool(name="ps", bufs=8, space="PSUM") as ps,
        ):
            vec_t = vecp.tile([128, 7 * KJ], f32, name="vec_t")

            def dec(j):
                return vec_t[:, j : j + 1]

            def omd(j):
                return vec_t[:, KJ + j : KJ + j + 1]

            def i1d(j):
                return vec_t[:, 2 * KJ + j : 2 * KJ + j + 1]

            def bsm(j):
                return vec_t[:, 3 * KJ + j : 3 * KJ + j + 1]

            def dv(j):
                return vec_t[:, 4 * KJ + j : 4 * KJ + j + 1]

            def om2(j):
                return vec_t[:, 5 * KJ + j : 5 * KJ + j + 1]

            def omdd(j):
                return vec_t[:, 6 * KJ + j : 6 * KJ + j + 1]

            A8 = [a8p.tile([128, K2, 2, 128], f8, name=f"a8_{j}") for j in range(KJ)]
            R8 = [r8p.tile([128, K2, 2, 128], f8, name=f"r8_{j}") for j in range(KJ)]
            H8P = h8pool.tile([128, K2, 2, B], f8, name="h8p")
            H8R = h8pool.tile([128, K2, 2, B], f8, name="h8r")
            C = [cp.tile([128, B], f32, name=f"c_{j}") for j in range(KJ)]
            ACC = [accp.tile([128, B], f32, name=f"acc_{j}") for j in range(KJ)]

            with tc.tile_pool(name="wihp", bufs=1) as wihp:
                WI = [wihp.tile([128, H], bf16, name=f"wih_{i}") for i in range(KI)]
                Xt = wihp.tile([128, KI, B], bf16, name="x_all")

                # DMA schedule: phase-0 operands (Xt, wih) are the early
                # critical path, interleaved in consumption order on the
                # two HW-DGE queues; h8/a8 follow; gpsimd (SWDGE) gets
                # wih0 + the latest-needed bulk (r8).
                # phase-0 consumes wih chunk i's columns 0:1024 in its
                # first two quarter-groups only, so each wih chunk is sent
                # as two column-halves with all low halves first: phase 0
                # can start on a 3MB working set instead of 5MB.
                HH = H // 2
                nc.gpsimd.dma_start(out=WI[0][:, 0:HH], in_=wih_r[0, :, 0:HH])
                nc.sync.dma_start(out=Xt[:, 0:2, :], in_=xt_r[:, 0:2, :])
                nc.scalar.dma_start(out=WI[1][:, 0:HH], in_=wih_r[1, :, 0:HH])
                nc.sync.dma_start(out=Xt[:, 2:4, :], in_=xt_r[:, 2:4, :])
                nc.scalar.dma_start(out=WI[2][:, 0:HH], in_=wih_r[2, :, 0:HH])
                nc.sync.dma_start(out=WI[3][:, 0:HH], in_=wih_r[3, :, 0:HH])
                nc.gpsimd.dma_start(out=vec_t[:], in_=vecs[:])
                nc.scalar.dma_start(out=Xt[:, 4:6, :], in_=xt_r[:, 4:6, :])
                nc.sync.dma_start(out=Xt[:, 6:8, :], in_=xt_r[:, 6:8, :])
                nc.scalar.dma_start(out=WI[4][:, 0:HH], in_=wih_r[4, :, 0:HH])
                nc.sync.dma_start(out=WI[5][:, 0:HH], in_=wih_r[5, :, 0:HH])
                nc.scalar.dma_start(out=WI[6][:, 0:HH], in_=wih_r[6, :, 0:HH])
                nc.sync.dma_start(out=WI[7][:, 0:HH], in_=wih_r[7, :, 0:HH])
                for i in range(KI):
                    q = nc.sync if i % 2 else nc.scalar
                    q.dma_start(out=WI[i][:, HH:], in_=wih_r[i, :, HH:])
                nc.gpsimd.dma_start(out=H8P[:], in_=h8p_r[:])
                nc.gpsimd.dma_start(out=A8[0][:], in_=a8_r[0])
                nc.scalar.dma_start(out=H8R[:], in_=h8r_r[:])
                for j in range(1, KJ):
                    q = nc.sync if j % 2 else nc.scalar
                    q.dma_start(out=A8[j][:], in_=a8_r[j])
                for j in range(KJ):
                    nc.gpsimd.dma_start(out=R8[j][:], in_=r8_r[j])

                # ---- phase 0: C_j = 2^s*(1-d)*(x @ W_ih.T + b_ih + b_hh),
                # transposed.  Four quarter-groups of 4 PSUM banks, so the
                # evacuations pipeline under the next quarter's matmuls and
                # round 0's banks free early.
                for jq in range(4):
                    psums = []
                    for jj in range(4):
                        p0 = ps.tile([128, B], f32, name=f"p0_{jq}_{jj}", tag="bank")
                        psums.append(p0)
                    for i in range(KI):
                        for jj in range(4):
                            j = jq * 4 + jj
                            nc.tensor.matmul(
                                psums[jj][:],
                                WI[i][:, j * 128 : (j + 1) * 128],
                                Xt[:, i, :],
                                start=(i == 0),
                                stop=(i == KI - 1),
                            )
                    for jj in range(4):
                        j = jq * 4 + jj
                        nc.vector.tensor_scalar(
                            out=C[j][:],
                            in0=psums[jj][:],
                            scalar1=bsm(j),
                            scalar2=om2(j),
                            op0=Alu.add,
                            op1=Alu.mult,
                        )

            with (
                tc.tile_pool(name="pp", bufs=1) as pp,
                tc.tile_pool(name="prp", bufs=1) as prp,
            ):
                P = [pp.tile([128, B], f32, name=f"p_{j}") for j in range(KJ)]
                PR = [
                    prp.tile([128, K2, 2, B], f8, name=f"pr_{b}") for b in range(2)
                ]

                def mm_group(bank, W, src, first, last, start_grp=True):
                    for k2 in range(K2):
                        nc.tensor.matmul(
                            bank[:],
                            W[:, k2],
                            src[:, k2],
                            start=(start_grp and first and k2 == 0),
                            stop=(last and k2 == K2 - 1),
                            perf_mode=DR,
                        )

                # C[j] is preloaded into each round's PSUM bank by ScalarE;
                # the round's matmuls then run with start=False and accumulate
                # on top (has_written bits are still set from the bank's
                # previous accumulation group, so the first matmul adds
                # rather than overwrites).  This removes the separate
                # "+C" DVE op per (round, j) — DVE was the saturated engine.
                pre_banks = {}

                def emit_preload(rr, cc):
                    bk = ps.tile([128, B], f32, name=f"bA_{rr}_{cc}", tag="bank")
                    nc.scalar.copy(out=bk[:], in_=C[cc][:])
                    pre_banks[(rr, cc)] = bk

                def epilogue0(j, bankA, bankB, OUT):
                    # P_0 = (A@q(h0) + A@q(h0-q(h0)) + zR0 + C) / (1-d)
                    zr = scr.tile([128, B], f32, name=f"zr_0_{j}", tag="s")
                    nc.vector.tensor_scalar_mul(
                        out=zr[:], in0=bankB[:], scalar1=2.0**-GAMMA
                    )
                    t1 = scr.tile([128, B], f32, name=f"t1_0_{j}", tag="s")
                    nc.vector.tensor_add(out=t1[:], in0=bankA[:], in1=zr[:])
                    t2 = scr.tile([128, B], f32, name=f"t2_0_{j}", tag="s")
                    nc.vector.tensor_add(out=t2[:], in0=t1[:], in1=C[j][:])
                    nc.vector.tensor_scalar_mul(out=P[j][:], in0=t2[:], scalar1=i1d(j))
                    uslab = OUT[:, j // 2, j % 2, :]
                    nc.scalar.activation(
                        out=uslab, in_=P[j][:], func=Tanh, scale=2.0**-S_POW
                    )
                    hs = scr.tile([128, B], f32, name=f"hs_{j}", tag="s")
                    nc.gpsimd.tensor_add(
                        out=hs[:],
                        in0=H8P[:, j // 2, j % 2, :],
                        in1=H8R[:, j // 2, j % 2, :],
                    )
                    am = scr.tile([128, B], f32, name=f"am_0_{j}", tag="s")
                    nc.scalar.mul(out=am[:], in_=hs[:], mul=dv(j))
                    nc.gpsimd.tensor_add(out=ACC[j][:], in0=am[:], in1=uslab)
                    if j >= KJ - 4:
                        # seed round 1's first 8 bank preloads near round-0's end
                        emit_preload(1, 2 * (j - (KJ - 4)))
                        emit_preload(1, 2 * (j - (KJ - 4)) + 1)

                def epilogue_steady(r, j, bank, OUT):
                    # P = d*P + (C + A@u)   [C was preloaded into the bank]
                    nc.vector.scalar_tensor_tensor(
                        out=P[j][:],
                        in0=P[j][:],
                        scalar=dec(j),
                        in1=bank[:],
                        op0=Alu.mult,
                        op1=Alu.add,
                    )
                    uslab = OUT[:, j // 2, j % 2, :]
                    nc.scalar.activation(
                        out=uslab, in_=P[j][:], func=Tanh, scale=2.0**-S_POW
                    )
                    if j < 8:
                        emit_preload(r, j + 8)
                    elif r < NUM_STEPS - 1:
                        emit_preload(r + 1, j - 8)
                    if r == NUM_STEPS - 1:
                        # final: hout = omd*u9 + (omd*d)*acc_8, staged in ACC[j]
                        am9 = scr.tile([128, B], f32, name=f"am9_{j}", tag="s")
                        if omdd_imm is not None:
                            nc.gpsimd.tensor_scalar_mul(
                                out=am9[:], in0=ACC[j][:], scalar1=omdd_imm
                            )
                        else:
                            nc.scalar.mul(out=am9[:], in_=ACC[j][:], mul=omdd(j))
                        nc.vector.scalar_tensor_tensor(
                            out=ACC[j][:],
                            in0=uslab,
                            scalar=omd(j),
                            in1=am9[:],
                            op0=Alu.mult,
                            op1=Alu.add,
                        )
                        q0, q1 = (
                            (nc.sync, nc.scalar),
                            (nc.scalar, nc.gpsimd),
                            (nc.gpsimd, nc.sync),
                        )[j % 3]
                        q0.dma_start(
                            out=ho_r[j, :, 0 : B // 2], in_=ACC[j][:, 0 : B // 2]
                        )
                        q1.dma_start(out=ho_r[j, :, B // 2 :], in_=ACC[j][:, B // 2 :])
                    else:
                        am = scr.tile([128, B], f32, name=f"am_{r}_{j}", tag="s")
                        nc.vector.tensor_scalar_mul(
                            out=am[:], in0=ACC[j][:], scalar1=dec(j)
                        )
                        nc.gpsimd.tensor_add(out=ACC[j][:], in0=am[:], in1=uslab)

                # ---- round 0: software-pipelined 4 deep so the h8r/a8 DMAs
                # hide behind the first h8p matmul groups.
                DEPTH = 4
                banksA = {}
                banksB = {}
                for j in range(KJ):
                    banksA[j] = ps.tile([128, B], f32, name=f"bA_0_{j}", tag="bank")
                    mm_group(banksA[j], A8[j], H8P, first=True, last=False)
                    if j >= DEPTH - 1:
                        jj = j - (DEPTH - 1)
                        mm_group(banksA[jj], A8[jj], H8R, first=False, last=True)
                        banksB[jj] = ps.tile(
                            [128, B], f32, name=f"bB_0_{jj}", tag="bank"
                        )
                        mm_group(banksB[jj], R8[jj], H8P, first=True, last=True)
                        epilogue0(jj, banksA[jj], banksB[jj], PR[1])
                for j in range(KJ - DEPTH + 1, KJ):
                    mm_group(banksA[j], A8[j], H8R, first=False, last=True)
                    banksB[j] = ps.tile([128, B], f32, name=f"bB_0_{j}", tag="bank")
                    mm_group(banksB[j], R8[j], H8P, first=True, last=True)
                    epilogue0(j, banksA[j], banksB[j], PR[1])

                # ---- rounds 1..9.  j=0/j=1's last k2 is deferred past
                # j=1's first chunks so the previous round's last tanh slabs
                # have ~3us of slack instead of ~1.5us.
                for r in range(1, NUM_STEPS):
                    IN = PR[r % 2]
                    OUT = PR[(r + 1) % 2]
                    b01 = {j: pre_banks.pop((r, j)) for j in (0, 1)}
                    for j in (0, 1):
                        for k2 in range(K2 - 1):
                            nc.tensor.matmul(
                                b01[j][:],
                                A8[j][:, k2],
                                IN[:, k2],
                                start=False,
                                stop=False,
                                perf_mode=DR,
                            )
                    for j in (0, 1):
                        nc.tensor.matmul(
                            b01[j][:],
                            A8[j][:, K2 - 1],
                            IN[:, K2 - 1],
                            start=False,
                            stop=True,
                            perf_mode=DR,
                        )
                        epilogue_steady(r, j, b01[j], OUT)
                    for j in range(2, KJ):
                        bank = pre_banks.pop((r, j))
                        mm_group(bank, A8[j], IN, first=True, last=True, start_grp=False)
                        epilogue_steady(r, j, bank, OUT)

    nc.compile()
    return nc


def _get_nc(omdd_imm=None):
    if omdd_imm is None and _NC_CACHE:
        # reuse whatever variant kernel() built (test harness timing path)
        return next(iter(_NC_CACHE.values()))
    key = ("nc", None if omdd_imm is None else round(omdd_imm, 9))
    if key not in _NC_CACHE:
        _NC_CACHE[key] = _build_nc(omdd_imm)
    return _NC_CACHE[key]


def _q8c(a):
    return np.clip(a, -240.0, 240.0).astype(ml_dtypes.float8_e4m3)


def _pack_w(M):
    # M [O=2048, I=2048]; out [p, j, k2, t, q] = M[j*128+q, (2*k2+t)*128+p]
    return np.ascontiguousarray(
        M.reshape(KJ, 128, K2, 2, 128).transpose(4, 0, 2, 3, 1).reshape(128, -1)
    )


def _host_prep(x, h0, W_ih, b_ih, W_hh, b_hh, tau):
    bf = ml_dtypes.bfloat16
    f32 = np.float32

    decay = np.exp(f32(-DT) / np.asarray(tau, f32)).astype(f32)
    omd = (f32(1.0) - decay).astype(f32)
    i1d = (f32(1.0) / omd).astype(f32)
    dv = (decay / omd).astype(f32)
    bsm = (np.asarray(b_ih, f32) + np.asarray(b_hh, f32)).astype(f32)
    om2 = (omd * f32(2.0**S_POW)).astype(f32)

    omdd = (omd * decay).astype(f32)
    vecs = np.zeros((128, 7 * KJ), f32)
    for g, v in enumerate((decay, omd, i1d, bsm, dv, om2, omdd)):
        vecs[:, g * KJ : (g + 1) * KJ] = v.reshape(KJ, 128).T

    wih_b = np.ascontiguousarray(np.asarray(W_ih, f32).T).astype(bf)  # [I, H]

    A = (f32(2.0**S_POW) * omd)[:, None] * np.asarray(W_hh, f32)
    a8_np = _pack_w(A)
    a8_q = _q8c(a8_np)
    Rp = (a8_np - a8_q.astype(f32)) * f32(2.0**GAMMA)
    r8_q = _q8c(Rp)

    in_maps = []
    for c in range(N_CORES):
        xs = np.asarray(x[c * B : (c + 1) * B], f32)
        hT = np.ascontiguousarray(np.asarray(h0[c * B : (c + 1) * B], f32).T)  # [H,B]
        xT = np.ascontiguousarray(xs.T).astype(bf)  # [I, B]
        h8p_q = _q8c(hT.reshape(K2, 2, 128, B).transpose(2, 0, 1, 3).reshape(128, -1))
        hres = hT - np.ascontiguousarray(
            h8p_q.astype(f32).reshape(128, K2, 2, B).transpose(1, 2, 0, 3).reshape(H, B)
        )
        h8r_q = _q8c(hres.reshape(K2, 2, 128, B).transpose(2, 0, 1, 3).reshape(128, -1))
        in_maps.append(
            {
                "x_t": xT,
                "wih": wih_b,
                "a8": a8_q,
                "r8": r8_q,
                "h8p": h8p_q,
                "h8r": h8r_q,
                "vecs": vecs,
            }
        )
    return in_maps


def kernel(x, h0, W_ih, b_ih, W_hh, b_hh, tau):
    from concourse.bass_utils import run_bass_kernel_spmd

    x, h0, W_ih, b_ih, W_hh, b_hh, tau = (
        np.asarray(a) for a in (x, h0, W_ih, b_ih, W_hh, b_hh, tau)
    )
    assert x.shape == (B_TOTAL, I) and h0.shape == (B_TOTAL, H)
    tau_f = np.asarray(tau, np.float32)
    if np.allclose(tau_f, tau_f.flat[0]):
        d0 = float(np.exp(np.float32(-DT) / tau_f.flat[0]))
        nc = _get_nc(omdd_imm=(1.0 - d0) * d0)
    else:
        nc = _get_nc()
    in_maps = _host_prep(x, h0, W_ih, b_ih, W_hh, b_hh, tau)
    res = run_bass_kernel_spmd(nc, in_maps, list(range(N_CORES)))
    out = np.empty((B_TOTAL, H), np.float32)
    for c in range(N_CORES):
        out[c * B : (c + 1) * B] = np.asarray(res.results[c]["hout"], np.float32).T
    return out



# revision 6
# speedup vs baseline: 1.2212x; 1.2212x over previous
"""Continuous-time RNN kernel for Trainium2 (8 NeuronCores, Bass/Tile).

Math (per reference):
    ih    = x @ W_ih.T + b_ih                     # time-invariant drive
    decay = exp(-dt / tau),  dt = 0.1
    10x:  h = decay * h + (1 - decay) * tanh(ih + h @ W_hh.T + b_hh)

Strategy (fp8 DoubleRow):
  - Data-parallel over batch: 4096 rows -> 8 cores x 512.
  - Reformulated recurrence in scaled-pre space (d scalar since tau==1):
        P_0     = (A@q(h0) + A@q(h0-q(h0)) + zR0 + C) * 1/(1-d)   [P = 2^s*pre]
        P_{r+1} = d*P_r + Cz + A@u_r
        u_r     = tanh(2^-s * P_r)    written directly as fp8 pairs by ACT
        acc_r   = d*acc_{r-1} + u_r   on gpsimd,  out = (1-d)*acc_9
    with A = 2^s*(1-d)*W_hh in fp8 e4m3 and a residual correction
    R = 2^g*(A - fp8(A)) in fp8 whose product R@moving is computed at
    rounds 0 and 1 only (u drifts ~8%/round so the cached correction
    stays accurate; round 0 matters most due to 1/(1-d) amplification)
    and folded into the additive constant C.
  - Matmuls use perf_mode=DoubleRow: contraction 256 per instruction
    (operands packed [128, 2, F] pair slabs), ~1.8x bf16 throughput.
  - Epilogue per (round, j): 2 DVE ops (scalar_tensor_tensor + add),
    1 ACT op (tanh, fp8 out), 1 gpsimd op (acc update) — under the
    PE's 8 matmuls (~1.73us).  Output staged in the dead P tiles.
  - Simulated end-to-end rel err ~5e-3 absmax (threshold 2e-2).
"""

import numpy as np
import ml_dtypes

H = 2048
I = 1024
B_TOTAL = 4096
N_CORES = 8
B = B_TOTAL // N_CORES  # 512 per-core batch shard
KJ = H // 128  # 16 output chunks of the hidden dim
K2 = KJ // 2  # 8 DoubleRow contraction pair-chunks
KI = I // 128  # 8 contraction chunks of the input dim
NUM_STEPS = 10
DT = 0.1
S_POW = 8  # weight scale 2^8
GAMMA = 5  # residual extra scale 2^5
REFRESH = (0,)  # rounds recomputing the weight-residual correction

_NC_CACHE = {}


def _build_nc():
    import concourse.mybir as mybir
    import concourse.tile as tile
    from concourse import bacc

    f32 = mybir.dt.float32
    bf16 = mybir.dt.bfloat16
    f8 = mybir.dt.float8e4
    Tanh = mybir.ActivationFunctionType.Tanh
    Alu = mybir.AluOpType
    DR = mybir.MatmulPerfMode.DoubleRow

    nc = bacc.Bacc(None, target_bir_lowering=False, debug=False)

    x_t = nc.declare_dram_parameter("x_t", [I, B], bf16, isOutput=False)
    wih = nc.declare_dram_parameter("wih", [I, H], bf16, isOutput=False)
    a8 = nc.declare_dram_parameter("a8", [128, KJ * K2 * 2 * 128], f8, isOutput=False)
    r8 = nc.declare_dram_parameter("r8", [128, KJ * K2 * 2 * 128], f8, isOutput=False)
    h8p = nc.declare_dram_parameter("h8p", [128, K2 * 2 * B], f8, isOutput=False)
    h8r = nc.declare_dram_parameter("h8r", [128, K2 * 2 * B], f8, isOutput=False)
    # per-partition vectors: [dec|omd|i1d|bsm|dv|om2|omdd], each [128, KJ]
    vecs = nc.declare_dram_parameter("vecs", [128, 7 * KJ], f32, isOutput=False)
    hout = nc.declare_dram_parameter("hout", [H, B], f32, isOutput=True)

    a8_r = a8[:].rearrange("p (j k two q) -> j p k two q", j=KJ, k=K2, two=2)
    r8_r = r8[:].rearrange("p (j k two q) -> j p k two q", j=KJ, k=K2, two=2)
    h8p_r = h8p[:].rearrange("p (k two b) -> p k two b", k=K2, two=2)
    h8r_r = h8r[:].rearrange("p (k two b) -> p k two b", k=K2, two=2)
    wih_r = wih[:].rearrange("(k p) j -> k p j", p=128)
    xt_r = x_t[:].rearrange("(i p) b -> p i b", p=128)  # [128, KI, B]
    ho_r = hout[:].rearrange("(k p) b -> k p b", p=128)

    with tile.TileContext(nc) as tc:
        with (
            tc.tile_pool(name="vecp", bufs=1) as vecp,
            tc.tile_pool(name="a8p", bufs=1) as a8p,
            tc.tile_pool(name="r8p", bufs=1) as r8p,
            tc.tile_pool(name="h8pool", bufs=1) as h8pool,
            tc.tile_pool(name="cp", bufs=1) as cp,
            tc.tile_pool(name="accp", bufs=1) as accp,
            tc.tile_pool(name="scr", bufs=4) as scr,
            tc.tile_pool(name="ps", bufs=8, space="PSUM") as ps,
        ):
            vec_t = vecp.tile([128, 7 * KJ], f32, name="vec_t")

            def dec(j):
                return vec_t[:, j : j + 1]

            def omd(j):
                return vec_t[:, KJ + j : KJ + j + 1]

            def i1d(j):
                return vec_t[:, 2 * KJ + j : 2 * KJ + j + 1]

            def bsm(j):
                return vec_t[:, 3 * KJ + j : 3 * KJ + j + 1]

            def dv(j):
                return vec_t[:, 4 * KJ + j : 4 * KJ + j + 1]

            def om2(j):
                return vec_t[:, 5 * KJ + j : 5 * KJ + j + 1]

            def omdd(j):
                return vec_t[:, 6 * KJ + j : 6 * KJ + j + 1]

            A8 = [a8p.tile([128, K2, 2, 128], f8, name=f"a8_{j}") for j in range(KJ)]
            R8 = [r8p.tile([128, K2, 2, 128], f8, name=f"r8_{j}") for j in range(KJ)]
            H8P = h8pool.tile([128, K2, 2, B], f8, name="h8p")
            H8R = h8pool.tile([128, K2, 2, B], f8, name="h8r")
            C = [cp.tile([128, B], f32, name=f"c_{j}") for j in range(KJ)]
            ACC = [accp.tile([128, B], f32, name=f"acc_{j}") for j in range(KJ)]

            with tc.tile_pool(name="wihp", bufs=1) as wihp:
                WI = [wihp.tile([128, H], bf16, name=f"wih_{i}") for i in range(KI)]
                Xt = wihp.tile([128, KI, B], bf16, name="x_all")

                # PE warmup: junk matmuls with no DMA dependency, emitted
                # first so the scheduler runs them during the initial DMA
                # wait.  Keeps the HAM activity window busy so phase 0
                # starts at the warm 2.4 GHz clock instead of paying
                # ~28 cold matmuls at 1.2 GHz (~6us), and bridges the
                # ~10us of PE idle before the first real operand lands.
                warm_w = wihp.tile([128, 128], bf16, name="warm_w")
                warm_m = wihp.tile([128, B], bf16, name="warm_m")
                nc.gpsimd.memset(warm_w[:], 0.0)
                nc.gpsimd.memset(warm_m[:], 0.0)
                warm_bank = ps.tile([128, B], f32, name="warm_bank", tag="bank")
                WARMUP = 28
                for w in range(WARMUP):
                    nc.tensor.matmul(
                        warm_bank[:],
                        warm_w[:],
                        warm_m[:],
                        start=(w == 0),
                        stop=(w == WARMUP - 1),
                    )

                # DMA schedule: phase-0 operands (Xt, wih) are the early
                # critical path, fed in exact consumption order on the two
                # HW-DGE queues (sync/scalar); gpsimd (SWDGE) carries the
                # small early pieces + the latest-needed bulk (r8).
                # phase 0's quarter-group jq consumes WI[i] columns
                # jq*512:(jq+1)*512 for all i, so wih is sent as column
                # quarters for jq=0/1 (first MM gated by only 0.5MB) and
                # halves after.
                HH = H // 2
                HQ = H // 4
                nc.sync.dma_start(out=Xt[:, 0:2, :], in_=xt_r[:, 0:2, :])
                nc.scalar.dma_start(out=WI[0][:, 0:HQ], in_=wih_r[0, :, 0:HQ])
                nc.gpsimd.dma_start(out=vec_t[:], in_=vecs[:])
                nc.gpsimd.dma_start(out=WI[1][:, 0:HQ], in_=wih_r[1, :, 0:HQ])
                nc.sync.dma_start(out=Xt[:, 2:4, :], in_=xt_r[:, 2:4, :])
                nc.scalar.dma_start(out=WI[2][:, 0:HQ], in_=wih_r[2, :, 0:HQ])
                nc.sync.dma_start(out=WI[3][:, 0:HQ], in_=wih_r[3, :, 0:HQ])
                nc.scalar.dma_start(out=Xt[:, 4:6, :], in_=xt_r[:, 4:6, :])
                nc.sync.dma_start(out=WI[4][:, 0:HQ], in_=wih_r[4, :, 0:HQ])
                nc.scalar.dma_start(out=Xt[:, 6:8, :], in_=xt_r[:, 6:8, :])
                nc.sync.dma_start(out=WI[5][:, 0:HQ], in_=wih_r[5, :, 0:HQ])
                nc.scalar.dma_start(out=WI[6][:, 0:HQ], in_=wih_r[6, :, 0:HQ])
                nc.sync.dma_start(out=WI[7][:, 0:HQ], in_=wih_r[7, :, 0:HQ])
                for i in range(KI):
                    q = nc.scalar if i % 2 else nc.sync
                    q.dma_start(out=WI[i][:, HQ:HH], in_=wih_r[i, :, HQ:HH])
                for i in range(KI):
                    q = nc.sync if i % 2 else nc.scalar
                    q.dma_start(out=WI[i][:, HH:], in_=wih_r[i, :, HH:])
                nc.gpsimd.dma_start(out=H8P[:], in_=h8p_r[:])
                nc.gpsimd.dma_start(out=A8[0][:], in_=a8_r[0])
                nc.scalar.dma_start(out=H8R[:], in_=h8r_r[:])
                for j in range(1, KJ):
                    q = nc.sync if j % 2 else nc.scalar
                    q.dma_start(out=A8[j][:], in_=a8_r[j])
                for j in range(KJ):
                    nc.gpsimd.dma_start(out=R8[j][:], in_=r8_r[j])

                # ---- phase 0: C_j = 2^s*(1-d)*(x @ W_ih.T + b_ih + b_hh),
                # transposed.  Four quarter-groups of 4 PSUM banks, so the
                # evacuations pipeline under the next quarter's matmuls and
                # round 0's banks free early.
                for jq in range(4):
                    psums = []
                    for jj in range(4):
                        p0 = ps.tile([128, B], f32, name=f"p0_{jq}_{jj}", tag="bank")
                        psums.append(p0)
                    for i in range(KI):
                        for jj in range(4):
                            j = jq * 4 + jj
                            nc.tensor.matmul(
                                psums[jj][:],
                                WI[i][:, j * 128 : (j + 1) * 128],
                                Xt[:, i, :],
                                start=(i == 0),
                                stop=(i == KI - 1),
                            )
                    for jj in range(4):
                        j = jq * 4 + jj
                        nc.vector.tensor_scalar(
                            out=C[j][:],
                            in0=psums[jj][:],
                            scalar1=bsm(j),
                            scalar2=om2(j),
                            op0=Alu.add,
                            op1=Alu.mult,
                        )

            with (
                tc.tile_pool(name="pp", bufs=1) as pp,
                tc.tile_pool(name="prp", bufs=1) as prp,
            ):
                P = [pp.tile([128, B], f32, name=f"p_{j}") for j in range(KJ)]
                PR = [
                    prp.tile([128, K2, 2, B], f8, name=f"pr_{b}") for b in range(2)
                ]

                def mm_group(bank, W, src, first, last, start_grp=True):
                    for k2 in range(K2):
                        nc.tensor.matmul(
                            bank[:],
                            W[:, k2],
                            src[:, k2],
                            start=(start_grp and first and k2 == 0),
                            stop=(last and k2 == K2 - 1),
                            perf_mode=DR,
                        )

                # C[j] is preloaded into each round's PSUM bank by ScalarE;
                # the round's matmuls then run with start=False and accumulate
                # on top (has_written bits are still set from the bank's
                # previous accumulation group, so the first matmul adds
                # rather than overwrites).  This removes the separate
                # "+C" DVE op per (round, j) — DVE was the saturated engine.
                pre_banks = {}

                def emit_preload(rr, cc):
                    bk = ps.tile([128, B], f32, name=f"bA_{rr}_{cc}", tag="bank")
                    nc.scalar.copy(out=bk[:], in_=C[cc][:])
                    pre_banks[(rr, cc)] = bk

                def epilogue0(j, bankA, bankB, OUT):
                    # P_0 = (A@q(h0) + A@q(h0-q(h0)) + zR0 + C) / (1-d)
                    zr = scr.tile([128, B], f32, name=f"zr_0_{j}", tag="s")
                    nc.vector.tensor_scalar_mul(
                        out=zr[:], in0=bankB[:], scalar1=2.0**-GAMMA
                    )
                    t1 = scr.tile([128, B], f32, name=f"t1_0_{j}", tag="s")
                    nc.vector.tensor_add(out=t1[:], in0=bankA[:], in1=zr[:])
                    t2 = scr.tile([128, B], f32, name=f"t2_0_{j}", tag="s")
                    nc.vector.tensor_add(out=t2[:], in0=t1[:], in1=C[j][:])
                    nc.vector.tensor_scalar_mul(out=P[j][:], in0=t2[:], scalar1=i1d(j))
                    uslab = OUT[:, j // 2, j % 2, :]
                    nc.scalar.activation(
                        out=uslab, in_=P[j][:], func=Tanh, scale=2.0**-S_POW
                    )
                    hs = scr.tile([128, B], f32, name=f"hs_{j}", tag="s")
                    nc.gpsimd.tensor_add(
                        out=hs[:],
                        in0=H8P[:, j // 2, j % 2, :],
                        in1=H8R[:, j // 2, j % 2, :],
                    )
                    am = scr.tile([128, B], f32, name=f"am_0_{j}", tag="s")
                    nc.scalar.mul(out=am[:], in_=hs[:], mul=dv(j))
                    nc.gpsimd.tensor_add(out=ACC[j][:], in0=am[:], in1=uslab)
                    if j >= KJ - 4:
                        # seed round 1's first 8 bank preloads near round-0's end
                        emit_preload(1, 2 * (j - (KJ - 4)))
                        emit_preload(1, 2 * (j - (KJ - 4)) + 1)

                def epilogue_steady(r, j, bank, OUT):
                    # P = d*P + (C + A@u)   [C was preloaded into the bank]
                    nc.vector.scalar_tensor_tensor(
                        out=P[j][:],
                        in0=P[j][:],
                        scalar=dec(j),
                        in1=bank[:],
                        op0=Alu.mult,
                        op1=Alu.add,
                    )
                    uslab = OUT[:, j // 2, j % 2, :]
                    nc.scalar.activation(
                        out=uslab, in_=P[j][:], func=Tanh, scale=2.0**-S_POW
                    )
                    if j < 8:
                        emit_preload(r, j + 8)
                    elif r < NUM_STEPS - 1:
                        emit_preload(r + 1, j - 8)
                    if r == NUM_STEPS - 1:
                        # final: hout = omd*u9 + (omd*d)*acc_8, staged in ACC[j]
                        # am9 runs on ACT (Scalar): gpsimd tensor_scalar_mul
                        # measures ~7.4us per [128,512] tile and its FIFO
                        # blocks the final DVE stt chain (~90us kernel tail).
                        am9 = scr.tile([128, B], f32, name=f"am9_{j}", tag="s")
                        nc.scalar.mul(out=am9[:], in_=ACC[j][:], mul=omdd(j))
                        nc.vector.scalar_tensor_tensor(
                            out=ACC[j][:],
                            in0=uslab,
                            scalar=omd(j),
                            in1=am9[:],
                            op0=Alu.mult,
                            op1=Alu.add,
                        )
                        q0, q1 = (
                            (nc.sync, nc.scalar),
                            (nc.scalar, nc.gpsimd),
                            (nc.gpsimd, nc.sync),
                        )[j % 3]
                        q0.dma_start(
                            out=ho_r[j, :, 0 : B // 2], in_=ACC[j][:, 0 : B // 2]
                        )
                        q1.dma_start(out=ho_r[j, :, B // 2 :], in_=ACC[j][:, B // 2 :])
                    else:
                        am = scr.tile([128, B], f32, name=f"am_{r}_{j}", tag="s")
                        nc.vector.tensor_scalar_mul(
                            out=am[:], in0=ACC[j][:], scalar1=dec(j)
                        )
                        nc.gpsimd.tensor_add(out=ACC[j][:], in0=am[:], in1=uslab)

                # ---- round 0: software-pipelined 4 deep so the h8r/a8 DMAs
                # hide behind the first h8p matmul groups.
                DEPTH = 4
                banksA = {}
                banksB = {}
                for j in range(KJ):
                    banksA[j] = ps.tile([128, B], f32, name=f"bA_0_{j}", tag="bank")
                    mm_group(banksA[j], A8[j], H8P, first=True, last=False)
                    if j >= DEPTH - 1:
                        jj = j - (DEPTH - 1)
                        mm_group(banksA[jj], A8[jj], H8R, first=False, last=True)
                        banksB[jj] = ps.tile(
                            [128, B], f32, name=f"bB_0_{jj}", tag="bank"
                        )
                        mm_group(banksB[jj], R8[jj], H8P, first=True, last=True)
                        epilogue0(jj, banksA[jj], banksB[jj], PR[1])
                for j in range(KJ - DEPTH + 1, KJ):
                    mm_group(banksA[j], A8[j], H8R, first=False, last=True)
                    banksB[j] = ps.tile([128, B], f32, name=f"bB_0_{j}", tag="bank")
                    mm_group(banksB[j], R8[j], H8P, first=True, last=True)
                    epilogue0(j, banksA[j], banksB[j], PR[1])

                # ---- rounds 1..9.  j=0/j=1's last k2 is deferred past
                # j=1's first chunks so the previous round's last tanh slabs
                # have ~3us of slack instead of ~1.5us.
                for r in range(1, NUM_STEPS):
                    IN = PR[r % 2]
                    OUT = PR[(r + 1) % 2]
                    b01 = {j: pre_banks.pop((r, j)) for j in (0, 1)}
                    for j in (0, 1):
                        for k2 in range(K2 - 1):
                            nc.tensor.matmul(
                                b01[j][:],
                                A8[j][:, k2],
                                IN[:, k2],
                                start=False,
                                stop=False,
                                perf_mode=DR,
                            )
                    for j in (0, 1):
                        nc.tensor.matmul(
                            b01[j][:],
                            A8[j][:, K2 - 1],
                            IN[:, K2 - 1],
                            start=False,
                            stop=True,
                            perf_mode=DR,
                        )
                        epilogue_steady(r, j, b01[j], OUT)
                    for j in range(2, KJ):
                        bank = pre_banks.pop((r, j))
                        mm_group(bank, A8[j], IN, first=True, last=True, start_grp=False)
                        epilogue_steady(r, j, bank, OUT)

    nc.compile()
    return nc


def _get_nc():
    if "nc" not in _NC_CACHE:
        _NC_CACHE["nc"] = _build_nc()
    return _NC_CACHE["nc"]


def _q8c(a):
    return np.clip(a, -240.0, 240.0).astype(ml_dtypes.float8_e4m3)


def _pack_w(M):
    # M [O=2048, I=2048]; out [p, j, k2, t, q] = M[j*128+q, (2*k2+t)*128+p]
    return np.ascontiguousarray(
        M.reshape(KJ, 128, K2, 2, 128).transpose(4, 0, 2, 3, 1).reshape(128, -1)
    )


def _host_prep(x, h0, W_ih, b_ih, W_hh, b_hh, tau):
    bf = ml_dtypes.bfloat16
    f32 = np.float32

    decay = np.exp(f32(-DT) / np.asarray(tau, f32)).astype(f32)
    omd = (f32(1.0) - decay).astype(f32)
    i1d = (f32(1.0) / omd).astype(f32)
    dv = (decay / omd).astype(f32)
    bsm = (np.asarray(b_ih, f32) + np.asarray(b_hh, f32)).astype(f32)
    om2 = (omd * f32(2.0**S_POW)).astype(f32)

    omdd = (omd * decay).astype(f32)
    vecs = np.zeros((128, 7 * KJ), f32)
    for g, v in enumerate((decay, omd, i1d, bsm, dv, om2, omdd)):
        vecs[:, g * KJ : (g + 1) * KJ] = v.reshape(KJ, 128).T

    wih_b = np.ascontiguousarray(np.asarray(W_ih, f32).T).astype(bf)  # [I, H]

    A = (f32(2.0**S_POW) * omd)[:, None] * np.asarray(W_hh, f32)
    a8_np = _pack_w(A)
    a8_q = _q8c(a8_np)
    Rp = (a8_np - a8_q.astype(f32)) * f32(2.0**GAMMA)
    r8_q = _q8c(Rp)

    in_maps = []
    for c in range(N_CORES):
        xs = np.asarray(x[c * B : (c + 1) * B], f32)
        hT = np.ascontiguousarray(np.asarray(h0[c * B : (c + 1) * B], f32).T)  # [H,B]
        xT = np.ascontiguousarray(xs.T).astype(bf)  # [I, B]
        h8p_q = _q8c(hT.reshape(K2, 2, 128, B).transpose(2, 0, 1, 3).reshape(128, -1))
        hres = hT - np.ascontiguousarray(
            h8p_q.astype(f32).reshape(128, K2, 2, B).transpose(1, 2, 0, 3).reshape(H, B)
        )
        h8r_q = _q8c(hres.reshape(K2, 2, 128, B).transpose(2, 0, 1, 3).reshape(128, -1))
        in_maps.append(
            {
                "x_t": xT,
                "wih": wih_b,
                "a8": a8_q,
                "r8": r8_q,
                "h8p": h8p_q,
                "h8r": h8r_q,
                "vecs": vecs,
            }
        )
    return in_maps


def kernel(x, h0, W_ih, b_ih, W_hh, b_hh, tau):
    from concourse.bass_utils import run_bass_kernel_spmd

    x, h0, W_ih, b_ih, W_hh, b_hh, tau = (
        np.asarray(a) for a in (x, h0, W_ih, b_ih, W_hh, b_hh, tau)
    )
    assert x.shape == (B_TOTAL, I) and h0.shape == (B_TOTAL, H)
    nc = _get_nc()
    in_maps = _host_prep(x, h0, W_ih, b_ih, W_hh, b_hh, tau)
    res = run_bass_kernel_spmd(nc, in_maps, list(range(N_CORES)))
    out = np.empty((B_TOTAL, H), np.float32)
    for c in range(N_CORES):
        out[c * B : (c + 1) * B] = np.asarray(res.results[c]["hout"], np.float32).T
    return out



# revision 9
# speedup vs baseline: 1.2335x; 1.0101x over previous
"""Continuous-time RNN kernel for Trainium2 (8 NeuronCores, Bass/Tile).

Math (per reference):
    ih    = x @ W_ih.T + b_ih                     # time-invariant drive
    decay = exp(-dt / tau),  dt = 0.1
    10x:  h = decay * h + (1 - decay) * tanh(ih + h @ W_hh.T + b_hh)

Strategy (fp8 DoubleRow):
  - Data-parallel over batch: 4096 rows -> 8 cores x 512.
  - Reformulated recurrence in scaled-pre space (d scalar since tau==1):
        P_0     = (A@q(h0) + A@q(h0-q(h0)) + zR0 + C) * 1/(1-d)   [P = 2^s*pre]
        P_{r+1} = d*P_r + Cz + A@u_r
        u_r     = tanh(2^-s * P_r)    written directly as fp8 pairs by ACT
        acc_r   = d*acc_{r-1} + u_r   on gpsimd,  out = (1-d)*acc_9
    with A = 2^s*(1-d)*W_hh in fp8 e4m3 and a residual correction
    R = 2^g*(A - fp8(A)) in fp8 whose product R@moving is computed at
    rounds 0 and 1 only (u drifts ~8%/round so the cached correction
    stays accurate; round 0 matters most due to 1/(1-d) amplification)
    and folded into the additive constant C.
  - Matmuls use perf_mode=DoubleRow: contraction 256 per instruction
    (operands packed [128, 2, F] pair slabs), ~1.8x bf16 throughput.
  - Epilogue per (round, j): 2 DVE ops (scalar_tensor_tensor + add),
    1 ACT op (tanh, fp8 out), 1 gpsimd op (acc update) — under the
    PE's 8 matmuls (~1.73us).  Output staged in the dead P tiles.
  - Simulated end-to-end rel err ~5e-3 absmax (threshold 2e-2).
"""

import numpy as np
import ml_dtypes

H = 2048
I = 1024
B_TOTAL = 4096
N_CORES = 8
B = B_TOTAL // N_CORES  # 512 per-core batch shard
KJ = H // 128  # 16 output chunks of the hidden dim
K2 = KJ // 2  # 8 DoubleRow contraction pair-chunks
KI = I // 128  # 8 contraction chunks of the input dim
NUM_STEPS = 10
DT = 0.1
S_POW = 8  # weight scale 2^8
GAMMA = 5  # residual extra scale 2^5
REFRESH = (0,)  # rounds recomputing the weight-residual correction

_NC_CACHE = {}


def _build_nc():
    import concourse.mybir as mybir
    import concourse.tile as tile
    from concourse import bacc

    f32 = mybir.dt.float32
    bf16 = mybir.dt.bfloat16
    f8 = mybir.dt.float8e4
    Tanh = mybir.ActivationFunctionType.Tanh
    Alu = mybir.AluOpType
    DR = mybir.MatmulPerfMode.DoubleRow

    nc = bacc.Bacc(None, target_bir_lowering=False, debug=False)

    x_t = nc.declare_dram_parameter("x_t", [I, B], bf16, isOutput=False)
    wih = nc.declare_dram_parameter("wih", [I, H], bf16, isOutput=False)
    a8 = nc.declare_dram_parameter("a8", [128, KJ * K2 * 2 * 128], f8, isOutput=False)
    r8 = nc.declare_dram_parameter("r8", [128, KJ * K2 * 2 * 128], f8, isOutput=False)
    h8p = nc.declare_dram_parameter("h8p", [128, K2 * 2 * B], f8, isOutput=False)
    h8r = nc.declare_dram_parameter("h8r", [128, K2 * 2 * B], f8, isOutput=False)
    # per-partition vectors: [dec|omd|i1d|bsm|dv|om2|omdd], each [128, KJ]
    vecs = nc.declare_dram_parameter("vecs", [128, 7 * KJ], f32, isOutput=False)
    hout = nc.declare_dram_parameter("hout", [H, B], f32, isOutput=True)

    a8_r = a8[:].rearrange("p (j k two q) -> j p k two q", j=KJ, k=K2, two=2)
    r8_r = r8[:].rearrange("p (j k two q) -> j p k two q", j=KJ, k=K2, two=2)
    h8p_r = h8p[:].rearrange("p (k two b) -> p k two b", k=K2, two=2)
    h8r_r = h8r[:].rearrange("p (k two b) -> p k two b", k=K2, two=2)
    wih_r = wih[:].rearrange("(k p) j -> k p j", p=128)
    xt_r = x_t[:].rearrange("(i p) b -> p i b", p=128)  # [128, KI, B]
    ho_r = hout[:].rearrange("(k p) b -> k p b", p=128)

    with tile.TileContext(nc) as tc:
        with (
            tc.tile_pool(name="vecp", bufs=1) as vecp,
            tc.tile_pool(name="a8p", bufs=1) as a8p,
            tc.tile_pool(name="r8p", bufs=1) as r8p,
            tc.tile_pool(name="h8pool", bufs=1) as h8pool,
            tc.tile_pool(name="cp", bufs=1) as cp,
            tc.tile_pool(name="accp", bufs=1) as accp,
            tc.tile_pool(name="scr", bufs=4) as scr,
            tc.tile_pool(name="ps", bufs=8, space="PSUM") as ps,
        ):
            vec_t = vecp.tile([128, 7 * KJ], f32, name="vec_t")

            def dec(j):
                return vec_t[:, j : j + 1]

            def omd(j):
                return vec_t[:, KJ + j : KJ + j + 1]

            def i1d(j):
                return vec_t[:, 2 * KJ + j : 2 * KJ + j + 1]

            def bsm(j):
                return vec_t[:, 3 * KJ + j : 3 * KJ + j + 1]

            def dv(j):
                return vec_t[:, 4 * KJ + j : 4 * KJ + j + 1]

            def om2(j):
                return vec_t[:, 5 * KJ + j : 5 * KJ + j + 1]

            def omdd(j):
                return vec_t[:, 6 * KJ + j : 6 * KJ + j + 1]

            A8 = [a8p.tile([128, K2, 2, 128], f8, name=f"a8_{j}") for j in range(KJ)]
            R8 = [r8p.tile([128, K2, 2, 128], f8, name=f"r8_{j}") for j in range(KJ)]
            H8P = h8pool.tile([128, K2, 2, B], f8, name="h8p")
            H8R = h8pool.tile([128, K2, 2, B], f8, name="h8r")
            C = [cp.tile([128, B], f32, name=f"c_{j}") for j in range(KJ)]
            ACC = [accp.tile([128, B], f32, name=f"acc_{j}") for j in range(KJ)]

            with tc.tile_pool(name="wihp", bufs=1) as wihp:
                WI = [wihp.tile([128, H], bf16, name=f"wih_{i}") for i in range(KI)]
                Xt = wihp.tile([128, KI, B], bf16, name="x_all")

                # PE warmup: junk matmuls with no DMA dependency, emitted
                # first so the scheduler runs them during the initial DMA
                # wait.  Keeps the HAM activity window busy so phase 0
                # starts at the warm 2.4 GHz clock instead of paying
                # ~28 cold matmuls at 1.2 GHz (~6us), and bridges the
                # ~10us of PE idle before the first real operand lands.
                warm_w = wihp.tile([128, 128], bf16, name="warm_w")
                warm_m = wihp.tile([128, B], bf16, name="warm_m")
                nc.gpsimd.memset(warm_w[:], 0.0)
                nc.gpsimd.memset(warm_m[:], 0.0)
                warm_bank = ps.tile([128, B], f32, name="warm_bank", tag="bank")
                WARMUP = 12
                for w in range(WARMUP):
                    nc.tensor.matmul(
                        warm_bank[:],
                        warm_w[:],
                        warm_m[:],
                        start=(w == 0),
                        stop=(w == WARMUP - 1),
                    )

                # DMA schedule: phase-0 operands (Xt, wih) are the early
                # critical path, fed in exact consumption order on the two
                # HW-DGE queues (sync/scalar); gpsimd (SWDGE) carries the
                # small early pieces + the latest-needed bulk (r8).
                # phase 0's quarter-group jq consumes WI[i] columns
                # jq*512:(jq+1)*512 for all i, so wih is sent as column
                # quarters for jq=0/1 (first MM gated by only 0.5MB) and
                # halves after.
                HH = H // 2
                HQ = H // 4
                nc.sync.dma_start(out=Xt[:, 0:2, :], in_=xt_r[:, 0:2, :])
                nc.scalar.dma_start(out=WI[0][:, 0:HQ], in_=wih_r[0, :, 0:HQ])
                nc.gpsimd.dma_start(out=vec_t[:], in_=vecs[:])
                nc.gpsimd.dma_start(out=WI[1][:, 0:HQ], in_=wih_r[1, :, 0:HQ])
                nc.sync.dma_start(out=Xt[:, 2:4, :], in_=xt_r[:, 2:4, :])
                nc.scalar.dma_start(out=WI[2][:, 0:HQ], in_=wih_r[2, :, 0:HQ])
                nc.sync.dma_start(out=WI[3][:, 0:HQ], in_=wih_r[3, :, 0:HQ])
                nc.scalar.dma_start(out=Xt[:, 4:6, :], in_=xt_r[:, 4:6, :])
                nc.sync.dma_start(out=WI[4][:, 0:HQ], in_=wih_r[4, :, 0:HQ])
                nc.scalar.dma_start(out=Xt[:, 6:8, :], in_=xt_r[:, 6:8, :])
                nc.sync.dma_start(out=WI[5][:, 0:HQ], in_=wih_r[5, :, 0:HQ])
                nc.scalar.dma_start(out=WI[6][:, 0:HQ], in_=wih_r[6, :, 0:HQ])
                nc.sync.dma_start(out=WI[7][:, 0:HQ], in_=wih_r[7, :, 0:HQ])
                for i in range(KI):
                    q = nc.scalar if i % 2 else nc.sync
                    q.dma_start(out=WI[i][:, HQ:HH], in_=wih_r[i, :, HQ:HH])
                for i in range(KI):
                    q = nc.sync if i % 2 else nc.scalar
                    q.dma_start(out=WI[i][:, HH:], in_=wih_r[i, :, HH:])
                nc.gpsimd.dma_start(out=H8P[:], in_=h8p_r[:])
                nc.gpsimd.dma_start(out=A8[0][:], in_=a8_r[0])
                nc.scalar.dma_start(out=H8R[:], in_=h8r_r[:])
                for j in range(1, KJ):
                    q = nc.sync if j % 2 else nc.scalar
                    q.dma_start(out=A8[j][:], in_=a8_r[j])
                for j in range(KJ):
                    nc.gpsimd.dma_start(out=R8[j][:], in_=r8_r[j])

                # ---- phase 0: C_j = 2^s*(1-d)*(x @ W_ih.T + b_ih + b_hh),
                # transposed.  Four quarter-groups of 4 PSUM banks, so the
                # evacuations pipeline under the next quarter's matmuls and
                # round 0's banks free early.
                for jq in range(4):
                    psums = []
                    for jj in range(4):
                        p0 = ps.tile([128, B], f32, name=f"p0_{jq}_{jj}", tag="bank")
                        psums.append(p0)
                    for i in range(KI):
                        for jj in range(4):
                            j = jq * 4 + jj
                            nc.tensor.matmul(
                                psums[jj][:],
                                WI[i][:, j * 128 : (j + 1) * 128],
                                Xt[:, i, :],
                                start=(i == 0),
                                stop=(i == KI - 1),
                            )
                    for jj in range(4):
                        j = jq * 4 + jj
                        nc.vector.tensor_scalar(
                            out=C[j][:],
                            in0=psums[jj][:],
                            scalar1=bsm(j),
                            scalar2=om2(j),
                            op0=Alu.add,
                            op1=Alu.mult,
                        )

            with (
                tc.tile_pool(name="pp", bufs=1) as pp,
                tc.tile_pool(name="prp", bufs=1) as prp,
            ):
                P = [pp.tile([128, B], f32, name=f"p_{j}") for j in range(KJ)]
                PR = [
                    prp.tile([128, K2, 2, B], f8, name=f"pr_{b}") for b in range(2)
                ]

                def mm_group(bank, W, src, first, last, start_grp=True):
                    for k2 in range(K2):
                        nc.tensor.matmul(
                            bank[:],
                            W[:, k2],
                            src[:, k2],
                            start=(start_grp and first and k2 == 0),
                            stop=(last and k2 == K2 - 1),
                            perf_mode=DR,
                        )

                # C[j] is preloaded into each round's PSUM bank by ScalarE;
                # the round's matmuls then run with start=False and accumulate
                # on top (has_written bits are still set from the bank's
                # previous accumulation group, so the first matmul adds
                # rather than overwrites).  This removes the separate
                # "+C" DVE op per (round, j) — DVE was the saturated engine.
                pre_banks = {}

                def emit_preload(rr, cc):
                    # Round 9's banks are preloaded with d*P8 + C (DVE stt)
                    # instead of C alone: after the matmuls accumulate A@u8
                    # the bank holds the full P9, so round 9 needs no P
                    # update at all and tanh reads the PSUM bank directly.
                    # Collapses the end-of-kernel dependency chain.
                    bk = ps.tile([128, B], f32, name=f"bA_{rr}_{cc}", tag="bank")
                    if rr == NUM_STEPS - 1:
                        nc.vector.scalar_tensor_tensor(
                            out=bk[:],
                            in0=P[cc][:],
                            scalar=dec(cc),
                            in1=C[cc][:],
                            op0=Alu.mult,
                            op1=Alu.add,
                        )
                    else:
                        nc.scalar.copy(out=bk[:], in_=C[cc][:])
                    pre_banks[(rr, cc)] = bk

                def epilogue0(j, bankA, bankB, OUT):
                    # P_0 = (A@q(h0) + A@q(h0-q(h0)) + zR0 + C) / (1-d)
                    zr = scr.tile([128, B], f32, name=f"zr_0_{j}", tag="s")
                    nc.vector.tensor_scalar_mul(
                        out=zr[:], in0=bankB[:], scalar1=2.0**-GAMMA
                    )
                    t1 = scr.tile([128, B], f32, name=f"t1_0_{j}", tag="s")
                    nc.vector.tensor_add(out=t1[:], in0=bankA[:], in1=zr[:])
                    t2 = scr.tile([128, B], f32, name=f"t2_0_{j}", tag="s")
                    nc.vector.tensor_add(out=t2[:], in0=t1[:], in1=C[j][:])
                    nc.vector.tensor_scalar_mul(out=P[j][:], in0=t2[:], scalar1=i1d(j))
                    uslab = OUT[:, j // 2, j % 2, :]
                    nc.scalar.activation(
                        out=uslab, in_=P[j][:], func=Tanh, scale=2.0**-S_POW
                    )
                    hs = scr.tile([128, B], f32, name=f"hs_{j}", tag="s")
                    nc.gpsimd.tensor_add(
                        out=hs[:],
                        in0=H8P[:, j // 2, j % 2, :],
                        in1=H8R[:, j // 2, j % 2, :],
                    )
                    am = scr.tile([128, B], f32, name=f"am_0_{j}", tag="s")
                    nc.scalar.mul(out=am[:], in_=hs[:], mul=dv(j))
                    nc.gpsimd.tensor_add(out=ACC[j][:], in0=am[:], in1=uslab)
                    if j >= KJ - 4:
                        # seed round 1's first 8 bank preloads near round-0's end
                        emit_preload(1, 2 * (j - (KJ - 4)))
                        emit_preload(1, 2 * (j - (KJ - 4)) + 1)

                def epilogue_steady(r, j, bank, OUT):
                    last = r == NUM_STEPS - 1
                    if not last:
                        # P = d*P + (C + A@u)   [C was preloaded into the bank]
                        nc.vector.scalar_tensor_tensor(
                            out=P[j][:],
                            in0=P[j][:],
                            scalar=dec(j),
                            in1=bank[:],
                            op0=Alu.mult,
                            op1=Alu.add,
                        )
                        uslab = OUT[:, j // 2, j % 2, :]
                        nc.scalar.activation(
                            out=uslab, in_=P[j][:], func=Tanh, scale=2.0**-S_POW
                        )
                    else:
                        # bank was preloaded with d*P8 + C, so it now holds
                        # the full P9: tanh straight off PSUM, u9 in f32.
                        u9 = scr.tile([128, B], f32, name=f"u9_{j}", tag="s")
                        nc.scalar.activation(
                            out=u9[:], in_=bank[:], func=Tanh, scale=2.0**-S_POW
                        )
                    if j < 8:
                        emit_preload(r, j + 8)
                    elif r < NUM_STEPS - 1:
                        emit_preload(r + 1, j - 8)
                    if last:
                        # final: hout = omd*u9 + (omd*d)*acc_8, staged in ACC[j]
                        # am9 runs on ACT (Scalar): gpsimd tensor_scalar_mul
                        # measures ~7.4us per [128,512] tile and its FIFO
                        # blocks the final DVE stt chain (~90us kernel tail).
                        am9 = scr.tile([128, B], f32, name=f"am9_{j}", tag="s")
                        nc.scalar.mul(out=am9[:], in_=ACC[j][:], mul=omdd(j))
                        nc.vector.scalar_tensor_tensor(
                            out=ACC[j][:],
                            in0=u9[:],
                            scalar=omd(j),
                            in1=am9[:],
                            op0=Alu.mult,
                            op1=Alu.add,
                        )
                        q0, q1 = (
                            (nc.sync, nc.scalar),
                            (nc.scalar, nc.gpsimd),
                            (nc.gpsimd, nc.sync),
                        )[j % 3]
                        q0.dma_start(
                            out=ho_r[j, :, 0 : B // 2], in_=ACC[j][:, 0 : B // 2]
                        )
                        q1.dma_start(out=ho_r[j, :, B // 2 :], in_=ACC[j][:, B // 2 :])
                    else:
                        # In round 8's second half the DVE also carries the
                        # round-9 bank preload stts, so shift the am mul to
                        # ACT there to keep DVE under the matmul budget.
                        am = scr.tile([128, B], f32, name=f"am_{r}_{j}", tag="s")
                        if r == NUM_STEPS - 2 and j >= 8:
                            nc.scalar.mul(out=am[:], in_=ACC[j][:], mul=dec(j))
                        else:
                            nc.vector.tensor_scalar_mul(
                                out=am[:], in0=ACC[j][:], scalar1=dec(j)
                            )
                        nc.gpsimd.tensor_add(out=ACC[j][:], in0=am[:], in1=uslab)

                # ---- round 0: software-pipelined 4 deep so the h8r/a8 DMAs
                # hide behind the first h8p matmul groups.
                DEPTH = 4
                banksA = {}
                banksB = {}
                for j in range(KJ):
                    banksA[j] = ps.tile([128, B], f32, name=f"bA_0_{j}", tag="bank")
                    mm_group(banksA[j], A8[j], H8P, first=True, last=False)
                    if j >= DEPTH - 1:
                        jj = j - (DEPTH - 1)
                        mm_group(banksA[jj], A8[jj], H8R, first=False, last=True)
                        banksB[jj] = ps.tile(
                            [128, B], f32, name=f"bB_0_{jj}", tag="bank"
                        )
                        mm_group(banksB[jj], R8[jj], H8P, first=True, last=True)
                        epilogue0(jj, banksA[jj], banksB[jj], PR[1])
                for j in range(KJ - DEPTH + 1, KJ):
                    mm_group(banksA[j], A8[j], H8R, first=False, last=True)
                    banksB[j] = ps.tile([128, B], f32, name=f"bB_0_{j}", tag="bank")
                    mm_group(banksB[j], R8[j], H8P, first=True, last=True)
                    epilogue0(j, banksA[j], banksB[j], PR[1])

                # ---- rounds 1..9.  j=0/j=1's last k2 is deferred past
                # j=1's first chunks so the previous round's last tanh slabs
                # have ~3us of slack instead of ~1.5us.
                for r in range(1, NUM_STEPS):
                    IN = PR[r % 2]
                    OUT = PR[(r + 1) % 2]
                    b01 = {j: pre_banks.pop((r, j)) for j in (0, 1)}
                    for j in (0, 1):
                        for k2 in range(K2 - 1):
                            nc.tensor.matmul(
                                b01[j][:],
                                A8[j][:, k2],
                                IN[:, k2],
                                start=False,
                                stop=False,
                                perf_mode=DR,
                            )
                    for j in (0, 1):
                        nc.tensor.matmul(
                            b01[j][:],
                            A8[j][:, K2 - 1],
                            IN[:, K2 - 1],
                            start=False,
                            stop=True,
                            perf_mode=DR,
                        )
                        epilogue_steady(r, j, b01[j], OUT)
                    for j in range(2, KJ):
                        bank = pre_banks.pop((r, j))
                        mm_group(bank, A8[j], IN, first=True, last=True, start_grp=False)
                        epilogue_steady(r, j, bank, OUT)

    nc.compile()
    return nc


def _get_nc():
    if "nc" not in _NC_CACHE:
        _NC_CACHE["nc"] = _build_nc()
    return _NC_CACHE["nc"]


def _q8c(a):
    return np.clip(a, -240.0, 240.0).astype(ml_dtypes.float8_e4m3)


def _pack_w(M):
    # M [O=2048, I=2048]; out [p, j, k2, t, q] = M[j*128+q, (2*k2+t)*128+p]
    return np.ascontiguousarray(
        M.reshape(KJ, 128, K2, 2, 128).transpose(4, 0, 2, 3, 1).reshape(128, -1)
    )


def _host_prep(x, h0, W_ih, b_ih, W_hh, b_hh, tau):
    bf = ml_dtypes.bfloat16
    f32 = np.float32

    decay = np.exp(f32(-DT) / np.asarray(tau, f32)).astype(f32)
    omd = (f32(1.0) - decay).astype(f32)
    i1d = (f32(1.0) / omd).astype(f32)
    dv = (decay / omd).astype(f32)
    bsm = (np.asarray(b_ih, f32) + np.asarray(b_hh, f32)).astype(f32)
    om2 = (omd * f32(2.0**S_POW)).astype(f32)

    omdd = (omd * decay).astype(f32)
    vecs = np.zeros((128, 7 * KJ), f32)
    for g, v in enumerate((decay, omd, i1d, bsm, dv, om2, omdd)):
        vecs[:, g * KJ : (g + 1) * KJ] = v.reshape(KJ, 128).T

    wih_b = np.ascontiguousarray(np.asarray(W_ih, f32).T).astype(bf)  # [I, H]

    A = (f32(2.0**S_POW) * omd)[:, None] * np.asarray(W_hh, f32)
    a8_np = _pack_w(A)
    a8_q = _q8c(a8_np)
    Rp = (a8_np - a8_q.astype(f32)) * f32(2.0**GAMMA)
    r8_q = _q8c(Rp)

    in_maps = []
    for c in range(N_CORES):
        xs = np.asarray(x[c * B : (c + 1) * B], f32)
        hT = np.ascontiguousarray(np.asarray(h0[c * B : (c + 1) * B], f32).T)  # [H,B]
        xT = np.ascontiguousarray(xs.T).astype(bf)  # [I, B]
        h8p_q = _q8c(hT.reshape(K2, 2, 128, B).transpose(2, 0, 1, 3).reshape(128, -1))
        hres = hT - np.ascontiguousarray(
            h8p_q.astype(f32).reshape(128, K2, 2, B).transpose(1, 2, 0, 3).reshape(H, B)
        )
        h8r_q = _q8c(hres.reshape(K2, 2, 128, B).transpose(2, 0, 1, 3).reshape(128, -1))
        in_maps.append(
            {
                "x_t": xT,
                "wih": wih_b,
                "a8": a8_q,
                "r8": r8_q,
                "h8p": h8p_q,
                "h8r": h8r_q,
                "vecs": vecs,
            }
        )
    return in_maps


def kernel(x, h0, W_ih, b_ih, W_hh, b_hh, tau):
    from concourse.bass_utils import run_bass_kernel_spmd

    x, h0, W_ih, b_ih, W_hh, b_hh, tau = (
        np.asarray(a) for a in (x, h0, W_ih, b_ih, W_hh, b_hh, tau)
    )
    assert x.shape == (B_TOTAL, I) and h0.shape == (B_TOTAL, H)
    nc = _get_nc()
    in_maps = _host_prep(x, h0, W_ih, b_ih, W_hh, b_hh, tau)
    res = run_bass_kernel_spmd(nc, in_maps, list(range(N_CORES)))
    out = np.empty((B_TOTAL, H), np.float32)
    for c in range(N_CORES):
        out[c * B : (c + 1) * B] = np.asarray(res.results[c]["hout"], np.float32).T
    return out

